# revision 8
# baseline (speedup 1.0000x reference)
"""Trainium2 Bass kernel for nn_Attention (B=4, N=2048, C=768, H=12, D=64).

Sharding: core c -> batch b=c//2, head-group hg=c%2 (6 heads each).
qkv_w column-parallel, proj_w row-parallel (host sums the 2 partials per b).
All matmuls in float32r (TF32-like, full PE rate for moving dims >= 256).
"""
import sys

sys.path.insert(0, "/opt/trn_rl_repo")

import numpy as np
import concourse.bass as bass
import concourse.mybir as mybir
import concourse.tile as tile
from concourse import bacc
from concourse.bass_utils import run_bass_kernel_spmd
from concourse.masks import make_identity

dt = mybir.dt
AF = mybir.ActivationFunctionType
ALU = mybir.AluOpType
AX = mybir.AxisListType

B, N, C = 4, 2048, 768
H, D = 12, 64
HPC = 6            # heads per core
EPS = 1e-6
NT = N // 128      # 16 token tiles
NCHUNK = C // 128  # 6 contraction chunks
SCALE = D ** -0.5  # 0.125

# Set True to round f32->f32r on device with DVE copies (if the BIR verifier
# rejects DMA-filled f32r tiles as matmul inputs).
ROUND_ON_DEVICE = False
DEBUG_DUMP = False


def _bc(ap, idx, count):
    """Insert a broadcast (step 0) free dim at position idx of an AP."""
    a = list(ap.ap)
    a.insert(idx, [0, count])
    return bass.AP(tensor=ap.tensor, offset=ap.offset, ap=a)


def build_program():
    nc = bacc.Bacc(None, target_bir_lowering=False)
    io_dt = dt.float32 if ROUND_ON_DEVICE else dt.float32r

    xT = nc.dram_tensor("xT", [C, N], io_dt, kind="ExternalInput")
    wqkvT = nc.dram_tensor("wqkvT", [C, 3 * HPC * D], io_dt, kind="ExternalInput")
    projT = nc.dram_tensor("projT", [HPC * D, C], io_dt, kind="ExternalInput")
    cwq = nc.dram_tensor("cwq", [N, D], dt.float32, kind="ExternalInput")
    swq = nc.dram_tensor("swq", [N, D], dt.float32, kind="ExternalInput")
    cwk = nc.dram_tensor("cwk", [N, D], dt.float32, kind="ExternalInput")
    swk = nc.dram_tensor("swk", [N, D], dt.float32, kind="ExternalInput")
    out = nc.dram_tensor("out", [N, C], dt.float32, kind="ExternalOutput")
    if DEBUG_DUMP:
        dbg_qT0 = nc.dram_tensor("dbg_qT0", [128, N], dt.float32, kind="ExternalOutput")
        dbg_kT0 = nc.dram_tensor("dbg_kT0", [128, N], dt.float32, kind="ExternalOutput")
        dbg_oT0 = nc.dram_tensor("dbg_oT0", [128, N], dt.float32, kind="ExternalOutput")
        dbg_vA = nc.dram_tensor("dbg_vA", [128, NT * HPC * (D + 1)], dt.float32, kind="ExternalOutput")

    with tile.TileContext(nc) as tc:
        with (
            tc.tile_pool(name="persist", bufs=1) as persist,
        ):
            # Persistent SBUF: qT/kT/oT head-pair tiles + augmented V.
            qT = [persist.tile([128, N], dt.float32r, name=f"qT{p}", tag=f"qT{p}") for p in range(3)]
            kT = [persist.tile([128, N], dt.float32r, name=f"kT{p}", tag=f"kT{p}") for p in range(3)]
            oT = [persist.tile([128, N], dt.float32r, name=f"oT{p}", tag=f"oT{p}") for p in range(3)]
            vA = persist.tile([128, NT, HPC, D + 1], dt.float32r, tag="vA")
            ident = persist.tile([128, 128], dt.float32, tag="ident")
            make_identity(nc, ident[:])
            ones1 = persist.tile([128, 1], dt.float32, tag="ones1")
            nc.vector.memset(ones1[:], 1.0)
            nc.vector.tensor_copy(vA[:, :, :, D : D + 1], _bc(_bc(ones1[:], 1, NT), 2, HPC))

            # ---------------- Phase 1: QKV + RMSNorm + RoPE + transpose ------
            with (
                tc.tile_pool(name="p1w", bufs=1) as p1w,
                tc.tile_pool(name="p1work", bufs=2) as p1work,
                tc.tile_pool(name="p1small", bufs=4) as p1small,
                tc.tile_pool(name="p1ps", bufs=2, space="PSUM") as p1ps,
                tc.tile_pool(name="p1pt", bufs=2, space="PSUM") as p1pt,
            ):
                # Weights / x^T (f32r). Loaded directly (or rounded via DVE).
                xr = []
                wr = []
                for j in range(NCHUNK):
                    xj = p1w.tile([128, N], dt.float32r, name=f"xr{j}", tag=f"xr{j}")
                    wj = p1w.tile([128, 3 * HPC * D], dt.float32r, name=f"wr{j}", tag=f"wr{j}")
                    if ROUND_ON_DEVICE:
                        xs = p1work.tile([128, N], dt.float32, tag="xstage")
                        ws = p1work.tile([128, 3 * HPC * D], dt.float32, tag="wstage")
                        nc.sync.dma_start(xs[:], xT[j * 128 : (j + 1) * 128, :])
                        nc.sync.dma_start(ws[:], wqkvT[j * 128 : (j + 1) * 128, :])
                        nc.vector.tensor_copy(xj[:], xs[:])
                        nc.vector.tensor_copy(wj[:], ws[:])
                    else:
                        nc.sync.dma_start(xj[:], xT[j * 128 : (j + 1) * 128, :])
                        nc.sync.dma_start(wj[:], wqkvT[j * 128 : (j + 1) * 128, :])
                    xr.append(xj)
                    wr.append(wj)

                epsb = p1w.tile([128, 1], dt.float32, name="epsb", tag="epsb")
                nc.vector.memset(epsb[:], float(D * EPS))

                # RoPE tables as [128 tok, NT, D]
                tabs = {}
                for name, dram in (("cwq", cwq), ("swq", swq), ("cwk", cwk), ("swk", swk)):
                    t = p1w.tile([128, NT, D], dt.float32, name=name, tag=name)
                    nc.sync.dma_start(t[:], dram.rearrange("(t p) d -> p t d", p=128))
                    tabs[name] = t

                for i in range(NT):
                    qp = p1ps.tile([128, HPC * D], dt.float32, tag="qp")
                    kp = p1ps.tile([128, HPC * D], dt.float32, tag="kp")
                    vp = p1ps.tile([128, HPC * D], dt.float32, tag="vp")
                    for j in range(NCHUNK):
                        xsl = xr[j][:, i * 128 : (i + 1) * 128]
                        st, sp = (j == 0), (j == NCHUNK - 1)
                        nc.tensor.matmul(qp[:], xsl, wr[j][:, 0 : HPC * D], start=st, stop=sp)
                        nc.tensor.matmul(kp[:], xsl, wr[j][:, HPC * D : 2 * HPC * D], start=st, stop=sp)
                        nc.tensor.matmul(vp[:], xsl, wr[j][:, 2 * HPC * D : 3 * HPC * D], start=st, stop=sp)

                    # V -> augmented SBUF tile (ones column pre-set)
                    nc.vector.tensor_copy(vA[:, i, :, 0:D], vp[:].rearrange("p (h d) -> p h d", h=HPC))

                    # RMSNorm + RoPE for q and k
                    for which, pp, cw, sw, dstT in (
                        ("q", qp, tabs["cwq"], tabs["swq"], qT),
                        ("k", kp, tabs["cwk"], tabs["swk"], kT),
                    ):
                        ph = pp[:].rearrange("p (h d) -> p h d", h=HPC)
                        sq = p1work.tile([128, HPC, D], dt.float32, tag="sq")
                        nc.scalar.activation(sq[:], ph, AF.Square)
                        ss = p1small.tile([128, HPC], dt.float32, tag="ss")
                        nc.vector.tensor_reduce(ss[:], sq[:], axis=AX.X, op=ALU.add)
                        nf = p1small.tile([128, HPC], dt.float32, tag="nf")
                        nc.scalar.activation(nf[:], ss[:], AF.Sqrt, bias=epsb[:])
                        nc.vector.reciprocal(nf[:], nf[:])
                        t_ = p1work.tile([128, HPC, D], dt.float32, tag="t_")
                        nc.vector.tensor_tensor(t_[:], ph, _bc(nf[:], 2, D), op=ALU.mult)
                        m1 = p1work.tile([128, HPC, D], dt.float32, tag="m1")
                        cwb = _bc(cw[:, i, :], 1, HPC)
                        swb = _bc(sw[:, i, :], 1, HPC)
                        nc.vector.tensor_tensor(m1[:], t_[:], cwb, op=ALU.mult)
                        m2 = p1work.tile([128, HPC, D], dt.float32, tag="m2")
                        h_ = D // 2
                        nc.vector.tensor_tensor(m2[:, :, 0:h_], t_[:, :, h_:D], swb[:, :, 0:h_], op=ALU.mult)
                        nc.vector.tensor_tensor(m2[:, :, h_:D], t_[:, :, 0:h_], swb[:, :, h_:D], op=ALU.mult)
                        qn = p1work.tile([128, HPC * D], dt.float32, tag="qn")
                        nc.vector.tensor_tensor(qn[:].rearrange("p (h d) -> p h d", h=HPC), m1[:], m2[:], op=ALU.add)
                        # transpose to feature-major [2h*64, tok]
                        for p in range(3):
                            tp = p1pt.tile([128, 128], dt.float32, tag="tp")
                            nc.tensor.transpose(tp[:], qn[:, p * 128 : (p + 1) * 128], ident[:])
                            nc.vector.tensor_copy(dstT[p][:, i * 128 : (i + 1) * 128], tp[:])

            if DEBUG_DUMP:
                nc.sync.dma_start(dbg_qT0[:], qT[0][:].bitcast(dt.float32))
                nc.sync.dma_start(dbg_kT0[:], kT[0][:].bitcast(dt.float32))
                nc.sync.dma_start(dbg_vA[:], vA[:].bitcast(dt.float32).rearrange("p a b c -> p (a b c)"))

            # ---------------- Phase 2: attention per head ---------------------
            with (
                tc.tile_pool(name="p2e", bufs=3) as p2e,
                tc.tile_pool(name="p2s", bufs=4) as p2s,
                tc.tile_pool(name="p2ps", bufs=2, space="PSUM") as p2ps,
                tc.tile_pool(name="p2pa", bufs=2, space="PSUM") as p2pa,
            ):
                for h in range(HPC):
                    pr = h // 2
                    off = 64 * (h % 2)
                    for g in range(4):  # qi groups of 512
                        av = p2pa.tile([65, 512], dt.float32, tag="av")
                        for kpair in range(8):
                            sp = p2ps.tile([128, 1024], dt.float32, tag="sp")
                            for half in range(2):
                                ki = kpair * 2 + half
                                nc.tensor.matmul(
                                    sp[:, half * 512 : (half + 1) * 512],
                                    kT[pr][off : off + 64, ki * 128 : (ki + 1) * 128],
                                    qT[pr][off : off + 64, g * 512 : (g + 1) * 512],
                                    start=True, stop=True,
                                )
                            es = p2e.tile([128, 1024], dt.float32r, tag="es")
                            nc.scalar.activation(es[:], sp[:], AF.Exp, scale=SCALE)
                            for half in range(2):
                                ki = kpair * 2 + half
                                nc.tensor.matmul(
                                    av[:],
                                    vA[:, ki, h, :],
                                    es[:, half * 512 : (half + 1) * 512],
                                    start=(ki == 0), stop=(ki == NT - 1),
                                )
                        rd = p2s.tile([1, 512], dt.float32, tag="rd")
                        nc.vector.reciprocal(rd[:], av[64:65, :])
                        bc = p2s.tile([64, 512], dt.float32, tag="bc")
                        nc.gpsimd.partition_broadcast(bc[:], rd[:], channels=64)
                        nc.vector.tensor_tensor(
                            oT[pr][off : off + 64, g * 512 : (g + 1) * 512],
                            av[0:64, :], bc[:], op=ALU.mult,
                        )

            if DEBUG_DUMP:
                nc.sync.dma_start(dbg_oT0[:], oT[0][:].bitcast(dt.float32))

            # ---------------- Phase 3: output projection ----------------------
            with (
                tc.tile_pool(name="p3w", bufs=1) as p3w,
                tc.tile_pool(name="p3o", bufs=3) as p3o,
                tc.tile_pool(name="p3ps", bufs=2, space="PSUM") as p3ps,
            ):
                prW = []
                for p in range(3):
                    wp = p3w.tile([128, C], dt.float32r, name=f"prW{p}", tag=f"prW{p}")
                    if ROUND_ON_DEVICE:
                        ws = p3o.tile([128, C], dt.float32, tag="pstage")
                        nc.sync.dma_start(ws[:], projT[p * 128 : (p + 1) * 128, :])
                        nc.vector.tensor_copy(wp[:], ws[:])
                    else:
                        nc.sync.dma_start(wp[:], projT[p * 128 : (p + 1) * 128, :])
                    prW.append(wp)
                for i in range(NT):
                    pp = p3ps.tile([128, C], dt.float32, tag="pp")
                    for p in range(3):
                        st, spp = (p == 0), (p == 2)
                        nc.tensor.matmul(pp[:, 0:512], oT[p][:, i * 128 : (i + 1) * 128], prW[p][:, 0:512], start=st, stop=spp)
                        nc.tensor.matmul(pp[:, 512:768], oT[p][:, i * 128 : (i + 1) * 128], prW[p][:, 512:768], start=st, stop=spp)
                    os_ = p3o.tile([128, C], dt.float32, tag="os")
                    nc.vector.tensor_copy(os_[:], pp[:])
                    nc.sync.dma_start(out[i * 128 : (i + 1) * 128, :], os_[:])

    nc.compile()
    return nc


_NC = None


def _get_nc():
    global _NC
    if _NC is None:
        _NC = build_program()
    return _NC


def _prep_inputs(x, cos, sin, qkv_w, q_norm_w, k_norm_w, proj_w):
    cos2 = np.asarray(cos, np.float32).reshape(N, D // 2)
    sin2 = np.asarray(sin, np.float32).reshape(N, D // 2)
    cos_full = np.concatenate([cos2, cos2], axis=1)          # [N, 64]
    sin_signed = np.concatenate([-sin2, sin2], axis=1)       # [N, 64]

    def tables(w):
        w = np.asarray(w, np.float32)
        wswap = np.concatenate([w[D // 2 :], w[: D // 2]])
        cw = (8.0 * cos_full * w[None, :]).astype(np.float32)
        sw = (8.0 * sin_signed * wswap[None, :]).astype(np.float32)
        return np.ascontiguousarray(cw), np.ascontiguousarray(sw)

    cwq_, swq_ = tables(q_norm_w)
    cwk_, swk_ = tables(k_norm_w)

    in_maps = []
    for c in range(8):
        b, hg = c // 2, c % 2
        h0 = HPC * hg
        rows = np.r_[h0 * D : (h0 + HPC) * D]
        wq = qkv_w[rows]
        wk = qkv_w[C + rows]
        wv = qkv_w[2 * C + rows]
        wqkvT_ = np.ascontiguousarray(np.concatenate([wq, wk, wv], 0).T, np.float32)
        projT_ = np.ascontiguousarray(proj_w[:, rows].T, np.float32)
        xT_ = np.ascontiguousarray(x[b].T, np.float32)
        in_maps.append({
            "xT": xT_, "wqkvT": wqkvT_, "projT": projT_,
            "cwq": cwq_, "swq": swq_, "cwk": cwk_, "swk": swk_,
        })
    return in_maps


def kernel(x, cos, sin, qkv_w, q_norm_w, k_norm_w, proj_w, proj_b, _want_trace=False):
    x = np.asarray(x, np.float32)
    qkv_w = np.asarray(qkv_w, np.float32)
    proj_w = np.asarray(proj_w, np.float32)
    proj_b = np.asarray(proj_b, np.float32)
    in_maps = _prep_inputs(x, cos, sin, qkv_w, q_norm_w, k_norm_w, proj_w)
    nc = _get_nc()
    res = run_bass_kernel_spmd(nc, in_maps, core_ids=list(range(8)), trace=_want_trace)
    out = np.empty((B, N, C), np.float32)
    for b in range(B):
        out[b] = res.results[2 * b]["out"] + res.results[2 * b + 1]["out"] + proj_b[None, :]
    if _want_trace:
        return out, res
    return out


# revision 33
# speedup vs baseline: 238.1996x; 238.1996x over previous
"""Trainium2 Bass kernel for nn_Attention (B=4, N=2048, C=768, H=12, D=64).

Sharding: core c -> batch b=c//2, head-group hg=c%2 (6 heads each).
qkv_w column-parallel, proj_w row-parallel (host sums the 2 partials per b).
All matmuls in float32r (TF32-like, full PE rate for moving dims >= 256).

Structure (single TileContext program per core), organized so attention for
head-pair p overlaps QK prep for pair p+1:
  - V matmuls for all token tiles first (vA tile, ones column folded in for
    the softmax denominator).
  - Per head-pair p: QK matmuls (host packs wqkvT as [q0k0|q1k1|q2k2|v] so
    each pair is a contiguous 256-wide moving operand), RMSNorm+RoPE via
    host-prepped tables (norm weights and the x8 factor folded in),
    PE-transpose to feature-major; then attention for that pair:
    S^T = K Q^T, exp on ACT (scale=1/8; no max-subtraction needed since
    RMSNorm gives ||q||=||k||=8 so |s|<=8), AV accumulation with the ones
    column giving the denominator; reciprocal + GPSIMD partition_broadcast
    to normalize.
  - Projection per token tile at the end (overlaps the attention tail).
"""
import sys

sys.path.insert(0, "/opt/trn_rl_repo")

import numpy as np
import concourse.bass as bass
import concourse.mybir as mybir
import concourse.tile as tile
from concourse import bacc
from concourse.bass_utils import run_bass_kernel_spmd
from concourse.masks import make_identity

dt = mybir.dt
AF = mybir.ActivationFunctionType
ALU = mybir.AluOpType
AX = mybir.AxisListType

B, N, C = 4, 2048, 768
H, D = 12, 64
HPC = 6            # heads per core
EPS = 1e-6
NT = N // 128      # 16 token tiles
NCHUNK = C // 128  # 6 contraction chunks
SCALE = D ** -0.5  # 0.125
NG = 4             # qi groups
G = N // NG        # 512 per group

DEBUG_DUMP = False


def _bc(ap, idx, count):
    """Insert a broadcast (step 0) free dim at position idx of an AP."""
    a = list(ap.ap)
    a.insert(idx, [0, count])
    return bass.AP(tensor=ap.tensor, offset=ap.offset, ap=a)


def build_program():
    nc = bacc.Bacc(None, target_bir_lowering=False)

    xT = nc.dram_tensor("xT", [C, N], dt.float32r, kind="ExternalInput")
    # host layout: [q0|k0 (256) | q1|k1 | q2|k2 | v (384)]
    wqkvT = nc.dram_tensor("wqkvT", [C, 3 * HPC * D], dt.float32r, kind="ExternalInput")
    projT = nc.dram_tensor("projT", [HPC * D, C], dt.float32r, kind="ExternalInput")
    cqk = nc.dram_tensor("cqk", [N, 2 * D], dt.float32, kind="ExternalInput")
    sqk = nc.dram_tensor("sqk", [N, 2 * D], dt.float32, kind="ExternalInput")
    out = nc.dram_tensor("out", [N, C], dt.float32, kind="ExternalOutput")
    if DEBUG_DUMP:
        dbg_qT0 = nc.dram_tensor("dbg_qT0", [128, N], dt.float32, kind="ExternalOutput")
        dbg_kT0 = nc.dram_tensor("dbg_kT0", [128, N], dt.float32, kind="ExternalOutput")
        dbg_oT0 = nc.dram_tensor("dbg_oT0", [128, N], dt.float32, kind="ExternalOutput")
        dbg_vA = nc.dram_tensor("dbg_vA", [128, NT * HPC * (D + 1)], dt.float32, kind="ExternalOutput")

    with tile.TileContext(nc) as tc:
        with (
            tc.tile_pool(name="persist", bufs=1) as persist,
            tc.tile_pool(name="qkrot", bufs=2) as qkrot,     # qT/kT rotate across pairs
            tc.tile_pool(name="work", bufs=2) as work,
            tc.tile_pool(name="qkblk", bufs=1) as qkblk,
            tc.tile_pool(name="tiny", bufs=2) as tiny,
            tc.tile_pool(name="den", bufs=1) as den,
            tc.tile_pool(name="p2e", bufs=2) as p2e,
            tc.tile_pool(name="psA", bufs=2, space="PSUM") as psA,   # qkv/tp/proj shared slots
            tc.tile_pool(name="psS", bufs=2, space="PSUM") as psS,   # scores (2 banks ea)
            tc.tile_pool(name="psV", bufs=2, space="PSUM") as psV,   # AV accum
        ):
            # ---------------- persistent tiles --------------------------------
            oT = [[persist.tile([128, G], dt.float32r, name=f"oT{p}_{g}", tag=f"oT{p}_{g}")
                   for g in range(NG)] for p in range(3)]
            vA = [persist.tile([128, 4, HPC, D + 1], dt.float32r, name=f"vA{kg}", tag=f"vA{kg}")
                  for kg in range(NG)]
            ident = persist.tile([128, 128], dt.float32, tag="ident")
            make_identity(nc, ident[:])
            ones1 = persist.tile([128, 1], dt.float32, tag="ones1")
            nc.vector.memset(ones1[:], 1.0)
            for kg in range(NG):
                nc.vector.tensor_copy(vA[kg][:, :, :, D : D + 1], _bc(_bc(ones1[:], 1, 4), 2, HPC))

            # weights / x^T / tables
            xw_cm = tc.tile_pool(name="xw", bufs=1)
            xw = xw_cm.__enter__()
            xr = []
            wr = []
            for j in range(NCHUNK):
                xj = xw.tile([128, N], dt.float32r, name=f"xr{j}", tag=f"xr{j}")
                wj = xw.tile([128, 3 * HPC * D], dt.float32r, name=f"wr{j}", tag=f"wr{j}")
                nc.sync.dma_start(xj[:], xT[j * 128 : (j + 1) * 128, :])
                nc.sync.dma_start(wj[:], wqkvT[j * 128 : (j + 1) * 128, :])
                xr.append(xj)
                wr.append(wj)
            tabs = {}
            for name, dram in (("cqk", cqk), ("sqk", sqk)):
                t = persist.tile([128, NT, 2, D], dt.float32, name=name, tag=name)
                nc.sync.dma_start(t[:], dram.rearrange("(t p) (qk d) -> p t qk d", p=128, qk=2))
                tabs[name] = t
            prW = []
            for p in range(3):
                wp = persist.tile([128, C], dt.float32r, name=f"prW{p}", tag=f"prW{p}")
                nc.sync.dma_start(wp[:], projT[p * 128 : (p + 1) * 128, :])
                prW.append(wp)

            # ---------------- per pair: QK prep then attention ----------------
            # (V matmuls interleaved into pair-0 prep so attention can start
            #  after the first 4-tile block)
            for p in range(3):
                qTp = [qkrot.tile([128, G], dt.float32r, name=f"qT{p}_{g}", tag=f"qT{g}") for g in range(NG)]
                kTp = [qkrot.tile([128, G], dt.float32r, name=f"kT{p}_{g}", tag=f"kT{g}") for g in range(NG)]
                # per-tile pipeline; transposes trail by 2 tiles so the PE
                # stream never waits on the DVE->Pool chain.
                pend = []

                def flush_one():
                    i, qn = pend.pop(0)
                    for half, dstT in ((0, qTp), (1, kTp)):
                        tp = psA.tile([128, 128], dt.float32, tag="qkv")
                        nc.tensor.transpose(tp[:], qn[:, half * 128 : (half + 1) * 128], ident[:])
                        dst = dstT[i // NG][:, (i % NG) * 128 : (i % NG + 1) * 128]
                        if p == 0:
                            nc.scalar.copy(dst, tp[:])
                        else:
                            nc.vector.tensor_copy(dst, tp[:])

                for i in range(NT):
                    if p == 0:
                        vp = psA.tile([128, HPC * D], dt.float32, tag="qkv")
                        for j in range(NCHUNK):
                            nc.tensor.matmul(vp[:], xr[j][:, i * 128 : (i + 1) * 128],
                                             wr[j][:, 6 * 128 : 6 * 128 + HPC * D],
                                             start=(j == 0), stop=(j == NCHUNK - 1))
                        nc.scalar.copy(vA[i // NG][:, i % NG, :, 0:D], vp[:].rearrange("p (h d) -> p h d", h=HPC))
                    qkp = psA.tile([128, 256], dt.float32, tag="qkv")
                    for j in range(NCHUNK):
                        nc.tensor.matmul(qkp[:], xr[j][:, i * 128 : (i + 1) * 128],
                                         wr[j][:, p * 256 : (p + 1) * 256],
                                         start=(j == 0), stop=(j == NCHUNK - 1))
                    if len(pend) >= 2:
                        flush_one()
                    qk_sb = qkblk.tile([128, 256], dt.float32, tag=f"qk_sb{i % 4}")
                    if p == 0:
                        nc.scalar.copy(qk_sb[:], qkp[:])
                    else:
                        nc.vector.tensor_copy(qk_sb[:], qkp[:])
                    qk4 = qk_sb[:].rearrange("p (h d) -> p h d", h=4)
                    sq = work.tile([128, 4, D], dt.float32, tag="m2")
                    nc.vector.tensor_tensor(sq[:], qk4, qk4, op=ALU.mult)
                    ss = tiny.tile([128, 4], dt.float32, tag="ss16")
                    nc.vector.tensor_reduce(ss[:], sq[:], axis=AX.X, op=ALU.add)
                    # rsqrt on DVE (bit-trick + 2 Newton): nf = 1/sqrt(ss+D*EPS)
                    ssh = tiny.tile([128, 4], dt.float32, tag="ssh")
                    nc.vector.tensor_scalar(ssh[:], ss[:], 0.5, 0.5 * D * EPS,
                                            op0=ALU.mult, op1=ALU.add)
                    y0i = tiny.tile([128, 4], dt.int32, tag="y0i")
                    nc.vector.tensor_scalar(y0i[:], ss[:].bitcast(dt.int32), 1, 0,
                                            op0=ALU.logical_shift_right, op1=ALU.bitwise_or)
                    nc.vector.tensor_scalar(y0i[:], y0i[:], -1, 0x5F3759DF,
                                            op0=ALU.mult, op1=ALU.add)
                    nf16 = tiny.tile([128, 4], dt.float32, tag="nf16")
                    y1 = tiny.tile([128, 4], dt.float32, tag="y1")
                    yw = tiny.tile([128, 4], dt.float32, tag="yw")
                    y = y0i[:].bitcast(dt.float32)
                    for dst_ in (y1, nf16):
                        nc.vector.tensor_tensor(yw[:], y, y, op=ALU.mult)
                        nc.vector.tensor_tensor(yw[:], yw[:], ssh[:], op=ALU.mult)
                        nc.vector.tensor_scalar(yw[:], yw[:], -1.0, 1.5,
                                                op0=ALU.mult, op1=ALU.add)
                        nc.vector.tensor_tensor(dst_[:], y, yw[:], op=ALU.mult)
                        y = dst_[:]
                    nfb = _bc(nf16[:], 2, D)
                    t_ = work.tile([128, 4, D], dt.float32, tag="t_")
                    nc.vector.tensor_tensor(t_[:], qk4, nfb, op=ALU.mult)
                    # tables: [128, NT, 2(qk), D] with heads broadcast
                    cwb = _bc(tabs["cqk"][:, i, :, :], 2, 2)
                    swb = _bc(tabs["sqk"][:, i, :, :], 2, 2)
                    t4 = t_[:].rearrange("p (qk h) d -> p qk h d", qk=2)
                    m1 = work.tile([128, 2, 2, D], dt.float32, tag="m1")
                    nc.vector.tensor_tensor(m1[:], t4, cwb, op=ALU.mult)
                    m2 = work.tile([128, 2, 2, D], dt.float32, tag="m2")
                    h_ = D // 2
                    nc.gpsimd.tensor_tensor(m2[:, :, :, 0:h_], t4[:, :, :, h_:D], swb[:, :, :, 0:h_], op=ALU.mult)
                    nc.gpsimd.tensor_tensor(m2[:, :, :, h_:D], t4[:, :, :, 0:h_], swb[:, :, :, h_:D], op=ALU.mult)
                    qn = work.tile([128, 256], dt.float32, tag="qn", bufs=4)
                    nc.gpsimd.tensor_tensor(qn[:].rearrange("p (qk h d) -> p qk h d", qk=2, h=2), m1[:], m2[:], op=ALU.add)
                    pend.append((i, qn))
                while pend:
                    flush_one()

                # attention for this pair
                for g in range(NG):
                    for hh in range(2):
                        h = 2 * p + hh
                        off = 64 * hh
                        av = psV.tile([65, G], dt.float32, tag="av")
                        for kpair in range(8):
                            sp = psS.tile([128, 1024], dt.float32, tag="sp")
                            for half in range(2):
                                ki = kpair * 2 + half
                                nc.tensor.matmul(
                                    sp[:, half * 512 : (half + 1) * 512],
                                    kTp[ki // NG][off : off + 64, (ki % NG) * 128 : (ki % NG + 1) * 128],
                                    qTp[g][off : off + 64, :],
                                    start=True, stop=True,
                                )
                            es = p2e.tile([128, 1024], dt.float32r, tag="es")
                            nc.scalar.activation(es[:], sp[:], AF.Exp, scale=SCALE)
                            for half in range(2):
                                ki = kpair * 2 + half
                                nc.tensor.matmul(
                                    av[:],
                                    vA[ki // NG][:, ki % NG, h, :],
                                    es[:, half * 512 : (half + 1) * 512],
                                    start=(ki == 0), stop=(ki == NT - 1),
                                )
                        rd = den.tile([1, G], dt.float32, tag="rd")
                        nc.vector.reciprocal(rd[:], av[64:65, :])
                        bc = den.tile([64, G], dt.float32, tag="bc")
                        nc.gpsimd.partition_broadcast(bc[:], rd[:], channels=64)
                        nc.vector.tensor_tensor(
                            oT[p][g][off : off + 64, :],
                            av[0:64, :], bc[:], op=ALU.mult,
                        )

            # ---------------- projection ---------------------------------------
            xw_cm.__exit__(None, None, None)
            outp_cm = tc.tile_pool(name="outp", bufs=2)
            outp = outp_cm.__enter__()
            for i in range(NT):
                p512 = psA.tile([128, 512], dt.float32, tag="qkv")
                p256 = psA.tile([128, 256], dt.float32, tag="qkv")
                for p in range(3):
                    st, spp = (p == 0), (p == 2)
                    sl = oT[p][i // NG][:, (i % NG) * 128 : (i % NG + 1) * 128]
                    nc.tensor.matmul(p512[:], sl, prW[p][:, 0:512], start=st, stop=spp)
                    nc.tensor.matmul(p256[:], sl, prW[p][:, 512:768], start=st, stop=spp)
                os_ = outp.tile([128, C], dt.float32, tag="os")
                nc.vector.tensor_copy(os_[:, 0:512], p512[:])
                nc.vector.tensor_copy(os_[:, 512:768], p256[:])
                nc.sync.dma_start(out[i * 128 : (i + 1) * 128, :], os_[:])
            outp_cm.__exit__(None, None, None)

            if DEBUG_DUMP:
                for g in range(NG):
                    nc.sync.dma_start(dbg_oT0[:, g * G : (g + 1) * G], oT[0][g][:].bitcast(dt.float32))
                for kg in range(NG):
                    nc.sync.dma_start(dbg_vA[:, kg * 4 * HPC * (D + 1) : (kg + 1) * 4 * HPC * (D + 1)],
                                      vA[kg][:].bitcast(dt.float32).rearrange("p a b c -> p (a b c)"))

    nc.compile()
    return nc


_NC = None


def _get_nc():
    global _NC
    if _NC is None:
        _NC = build_program()
    return _NC


def _prep_inputs(x, cos, sin, qkv_w, q_norm_w, k_norm_w, proj_w):
    cos2 = np.asarray(cos, np.float32).reshape(N, D // 2)
    sin2 = np.asarray(sin, np.float32).reshape(N, D // 2)
    cos_full = np.concatenate([cos2, cos2], axis=1)          # [N, 64]
    sin_signed = np.concatenate([-sin2, sin2], axis=1)       # [N, 64]

    def tables(w):
        w = np.asarray(w, np.float32)
        wswap = np.concatenate([w[D // 2 :], w[: D // 2]])
        cw = (8.0 * cos_full * w[None, :]).astype(np.float32)
        sw = (8.0 * sin_signed * wswap[None, :]).astype(np.float32)
        return np.ascontiguousarray(cw), np.ascontiguousarray(sw)

    cwq_, swq_ = tables(q_norm_w)
    cwk_, swk_ = tables(k_norm_w)
    cqk_ = np.ascontiguousarray(np.stack([cwq_, cwk_], axis=1).reshape(N, 2 * D))
    sqk_ = np.ascontiguousarray(np.stack([swq_, swk_], axis=1).reshape(N, 2 * D))

    in_maps = []
    for c in range(8):
        b, hg = c // 2, c % 2
        h0 = HPC * hg
        rows = np.r_[h0 * D : (h0 + HPC) * D]
        wq = qkv_w[rows]          # [384, C]
        wk = qkv_w[C + rows]
        wv = qkv_w[2 * C + rows]
        # pack as [q0|k0, q1|k1, q2|k2, v]
        parts = []
        for p in range(3):
            parts.append(wq[p * 128 : (p + 1) * 128])
            parts.append(wk[p * 128 : (p + 1) * 128])
        parts.append(wv)
        wqkvT_ = np.ascontiguousarray(np.concatenate(parts, 0).T, np.float32)
        projT_ = np.ascontiguousarray(proj_w[:, rows].T, np.float32)
        xT_ = np.ascontiguousarray(x[b].T, np.float32)
        in_maps.append({
            "xT": xT_, "wqkvT": wqkvT_, "projT": projT_,
            "cqk": cqk_, "sqk": sqk_,
        })
    return in_maps


def kernel(x, cos, sin, qkv_w, q_norm_w, k_norm_w, proj_w, proj_b, _want_trace=False):
    x = np.asarray(x, np.float32)
    qkv_w = np.asarray(qkv_w, np.float32)
    proj_w = np.asarray(proj_w, np.float32)
    proj_b = np.asarray(proj_b, np.float32)
    in_maps = _prep_inputs(x, cos, sin, qkv_w, q_norm_w, k_norm_w, proj_w)
    nc = _get_nc()
    res = run_bass_kernel_spmd(nc, in_maps, core_ids=list(range(8)), trace=_want_trace)
    out = np.empty((B, N, C), np.float32)
    for b in range(B):
        out[b] = res.results[2 * b]["out"] + res.results[2 * b + 1]["out"] + proj_b[None, :]
    if _want_trace:
        return out, res
    return out


# revision 36
# speedup vs baseline: 238.5076x; 1.0013x over previous
"""Trainium2 Bass kernel for nn_Attention (B=4, N=2048, C=768, H=12, D=64).

Sharding: core c -> batch b=c//2, head-group hg=c%2 (6 heads each).
qkv_w column-parallel, proj_w row-parallel (host sums the 2 partials per b).
All matmuls in float32r (TF32-like, full PE rate for moving dims >= 256).

Structure (single TileContext program per core), organized so attention for
head-pair p overlaps QK prep for pair p+1:
  - V matmuls for all token tiles first (vA tile, ones column folded in for
    the softmax denominator).
  - Per head-pair p: QK matmuls (host packs wqkvT as [q0k0|q1k1|q2k2|v] so
    each pair is a contiguous 256-wide moving operand), RMSNorm+RoPE via
    host-prepped tables (norm weights and the x8 factor folded in),
    PE-transpose to feature-major; then attention for that pair:
    S^T = K Q^T, exp on ACT (scale=1/8; no max-subtraction needed since
    RMSNorm gives ||q||=||k||=8 so |s|<=8), AV accumulation with the ones
    column giving the denominator; reciprocal + GPSIMD partition_broadcast
    to normalize.
  - Projection per token tile at the end (overlaps the attention tail).
"""
import sys

sys.path.insert(0, "/opt/trn_rl_repo")

import numpy as np
import concourse.bass as bass
import concourse.mybir as mybir
import concourse.tile as tile
from concourse import bacc
from concourse.bass_utils import run_bass_kernel_spmd
from concourse.masks import make_identity

dt = mybir.dt
AF = mybir.ActivationFunctionType
ALU = mybir.AluOpType
AX = mybir.AxisListType

B, N, C = 4, 2048, 768
H, D = 12, 64
HPC = 6            # heads per core
EPS = 1e-6
NT = N // 128      # 16 token tiles
NCHUNK = C // 128  # 6 contraction chunks
SCALE = D ** -0.5  # 0.125
NG = 4             # qi groups
G = N // NG        # 512 per group

DEBUG_DUMP = False


def _bc(ap, idx, count):
    """Insert a broadcast (step 0) free dim at position idx of an AP."""
    a = list(ap.ap)
    a.insert(idx, [0, count])
    return bass.AP(tensor=ap.tensor, offset=ap.offset, ap=a)


def build_program():
    nc = bacc.Bacc(None, target_bir_lowering=False)

    xT = nc.dram_tensor("xT", [C, N], dt.float32r, kind="ExternalInput")
    # host layout: [q0|k0 (256) | q1|k1 | q2|k2 | v (384)]
    wqkvT = nc.dram_tensor("wqkvT", [C, 3 * HPC * D], dt.float32r, kind="ExternalInput")
    projT = nc.dram_tensor("projT", [HPC * D, C], dt.float32r, kind="ExternalInput")
    cqk = nc.dram_tensor("cqk", [N, 2 * D], dt.float32, kind="ExternalInput")
    sqk = nc.dram_tensor("sqk", [N, 2 * D], dt.float32, kind="ExternalInput")
    out = nc.dram_tensor("out", [N, C], dt.float32, kind="ExternalOutput")
    if DEBUG_DUMP:
        dbg_qT0 = nc.dram_tensor("dbg_qT0", [128, N], dt.float32, kind="ExternalOutput")
        dbg_kT0 = nc.dram_tensor("dbg_kT0", [128, N], dt.float32, kind="ExternalOutput")
        dbg_oT0 = nc.dram_tensor("dbg_oT0", [128, N], dt.float32, kind="ExternalOutput")
        dbg_vA = nc.dram_tensor("dbg_vA", [128, NT * HPC * (D + 1)], dt.float32, kind="ExternalOutput")

    with tile.TileContext(nc) as tc:
        with (
            tc.tile_pool(name="persist", bufs=1) as persist,
            tc.tile_pool(name="qkrot", bufs=2) as qkrot,     # qT/kT rotate across pairs
            tc.tile_pool(name="work", bufs=2) as work,
            tc.tile_pool(name="qkblk", bufs=1) as qkblk,
            tc.tile_pool(name="tiny", bufs=2) as tiny,
            tc.tile_pool(name="den", bufs=1) as den,
            tc.tile_pool(name="p2e", bufs=2) as p2e,
            tc.tile_pool(name="psA", bufs=2, space="PSUM") as psA,   # qkv/tp/proj shared slots
            tc.tile_pool(name="psS", bufs=2, space="PSUM") as psS,   # scores (2 banks ea)
            tc.tile_pool(name="psV", bufs=2, space="PSUM") as psV,   # AV accum
        ):
            # ---------------- persistent tiles --------------------------------
            oT = [[persist.tile([128, G], dt.float32r, name=f"oT{p}_{g}", tag=f"oT{p}_{g}")
                   for g in range(NG)] for p in range(3)]
            vA = [persist.tile([128, 4, HPC, D + 1], dt.float32r, name=f"vA{kg}", tag=f"vA{kg}")
                  for kg in range(NG)]
            ident = persist.tile([128, 128], dt.float32, tag="ident")
            make_identity(nc, ident[:])
            ones1 = persist.tile([128, 1], dt.float32, tag="ones1")
            nc.vector.memset(ones1[:], 1.0)
            for kg in range(NG):
                nc.vector.tensor_copy(vA[kg][:, :, :, D : D + 1], _bc(_bc(ones1[:], 1, 4), 2, HPC))

            # weights / x^T / tables
            xw_cm = tc.tile_pool(name="xw", bufs=1)
            xw = xw_cm.__enter__()
            xr = []
            wr = []
            for j in range(NCHUNK):
                xj = xw.tile([128, N], dt.float32r, name=f"xr{j}", tag=f"xr{j}")
                wj = xw.tile([128, 3 * HPC * D], dt.float32r, name=f"wr{j}", tag=f"wr{j}")
                nc.sync.dma_start(xj[:], xT[j * 128 : (j + 1) * 128, :])
                nc.sync.dma_start(wj[:], wqkvT[j * 128 : (j + 1) * 128, :])
                xr.append(xj)
                wr.append(wj)
            tabs = {}
            for name, dram in (("cqk", cqk), ("sqk", sqk)):
                t = persist.tile([128, NT, 2, D], dt.float32, name=name, tag=name)
                nc.sync.dma_start(t[:], dram.rearrange("(t p) (qk d) -> p t qk d", p=128, qk=2))
                tabs[name] = t
            prW = []
            for p in range(3):
                wp = persist.tile([128, C], dt.float32r, name=f"prW{p}", tag=f"prW{p}")
                nc.sync.dma_start(wp[:], projT[p * 128 : (p + 1) * 128, :])
                prW.append(wp)

            # ------- interleaved emission: prep / attention / projection ------
            # Engines execute their instruction streams in order, so emission
            # order IS the schedule. Pair p's attention units interleave the
            # prep-tile emission for pair p+1 (2 tiles per unit) so the PE
            # stream mixes prep matmuls with scores/AV instead of bunching
            # them at pair boundaries. Pair-2 attention interleaves the
            # projection of already-finished qi groups.

            def new_pair_state(p):
                return {
                    "p": p,
                    "qT": [qkrot.tile([128, G], dt.float32r, name=f"qT{p}_{g}", tag=f"qT{g}") for g in range(NG)],
                    "kT": [qkrot.tile([128, G], dt.float32r, name=f"kT{p}_{g}", tag=f"kT{g}") for g in range(NG)],
                    "pend": [],
                    "next": 0,
                }

            def flush_one(st):
                i, qn = st["pend"].pop(0)
                for half, dstT in ((0, st["qT"]), (1, st["kT"])):
                    tp = psA.tile([128, 128], dt.float32, tag="qkv")
                    nc.tensor.transpose(tp[:], qn[:, half * 128 : (half + 1) * 128], ident[:])
                    dst = dstT[i // NG][:, (i % NG) * 128 : (i % NG + 1) * 128]
                    if st["p"] == 0:
                        nc.scalar.copy(dst, tp[:])
                    else:
                        nc.vector.tensor_copy(dst, tp[:])

            def emit_prep_tile(st):
                p = st["p"]
                i = st["next"]
                st["next"] += 1
                if p == 0:
                    vp = psA.tile([128, HPC * D], dt.float32, tag="qkv")
                    for j in range(NCHUNK):
                        nc.tensor.matmul(vp[:], xr[j][:, i * 128 : (i + 1) * 128],
                                         wr[j][:, 6 * 128 : 6 * 128 + HPC * D],
                                         start=(j == 0), stop=(j == NCHUNK - 1))
                    nc.scalar.copy(vA[i // NG][:, i % NG, :, 0:D], vp[:].rearrange("p (h d) -> p h d", h=HPC))
                qkp = psA.tile([128, 256], dt.float32, tag="qkv")
                for j in range(NCHUNK):
                    nc.tensor.matmul(qkp[:], xr[j][:, i * 128 : (i + 1) * 128],
                                     wr[j][:, p * 256 : (p + 1) * 256],
                                     start=(j == 0), stop=(j == NCHUNK - 1))
                if len(st["pend"]) >= 2:
                    flush_one(st)
                qk_sb = qkblk.tile([128, 256], dt.float32, tag=f"qk_sb{i % 4}")
                if p == 0:
                    nc.scalar.copy(qk_sb[:], qkp[:])
                else:
                    nc.vector.tensor_copy(qk_sb[:], qkp[:])
                qk4 = qk_sb[:].rearrange("p (h d) -> p h d", h=4)
                sq = work.tile([128, 4, D], dt.float32, tag="m2")
                nc.vector.tensor_tensor(sq[:], qk4, qk4, op=ALU.mult)
                ss = tiny.tile([128, 4], dt.float32, tag="ss16")
                nc.vector.tensor_reduce(ss[:], sq[:], axis=AX.X, op=ALU.add)
                # rsqrt on DVE (bit-trick + 2 Newton): nf = 1/sqrt(ss+D*EPS)
                ssh = tiny.tile([128, 4], dt.float32, tag="ssh")
                nc.vector.tensor_scalar(ssh[:], ss[:], 0.5, 0.5 * D * EPS,
                                        op0=ALU.mult, op1=ALU.add)
                y0i = tiny.tile([128, 4], dt.int32, tag="y0i")
                nc.vector.tensor_scalar(y0i[:], ss[:].bitcast(dt.int32), 1, 0,
                                        op0=ALU.logical_shift_right, op1=ALU.bitwise_or)
                nc.vector.tensor_scalar(y0i[:], y0i[:], -1, 0x5F3759DF,
                                        op0=ALU.mult, op1=ALU.add)
                nf16 = tiny.tile([128, 4], dt.float32, tag="nf16")
                y1 = tiny.tile([128, 4], dt.float32, tag="y1")
                yw = tiny.tile([128, 4], dt.float32, tag="yw")
                y = y0i[:].bitcast(dt.float32)
                for dst_ in (y1, nf16):
                    nc.vector.tensor_tensor(yw[:], y, y, op=ALU.mult)
                    nc.vector.tensor_tensor(yw[:], yw[:], ssh[:], op=ALU.mult)
                    nc.vector.tensor_scalar(yw[:], yw[:], -1.0, 1.5,
                                            op0=ALU.mult, op1=ALU.add)
                    nc.vector.tensor_tensor(dst_[:], y, yw[:], op=ALU.mult)
                    y = dst_[:]
                nfb = _bc(nf16[:], 2, D)
                t_ = work.tile([128, 4, D], dt.float32, tag="t_")
                nc.vector.tensor_tensor(t_[:], qk4, nfb, op=ALU.mult)
                # tables: [128, NT, 2(qk), D] with heads broadcast
                cwb = _bc(tabs["cqk"][:, i, :, :], 2, 2)
                swb = _bc(tabs["sqk"][:, i, :, :], 2, 2)
                t4 = t_[:].rearrange("p (qk h) d -> p qk h d", qk=2)
                m1 = work.tile([128, 2, 2, D], dt.float32, tag="m1")
                nc.vector.tensor_tensor(m1[:], t4, cwb, op=ALU.mult)
                m2 = work.tile([128, 2, 2, D], dt.float32, tag="m2")
                h_ = D // 2
                nc.gpsimd.tensor_tensor(m2[:, :, :, 0:h_], t4[:, :, :, h_:D], swb[:, :, :, 0:h_], op=ALU.mult)
                nc.gpsimd.tensor_tensor(m2[:, :, :, h_:D], t4[:, :, :, 0:h_], swb[:, :, :, h_:D], op=ALU.mult)
                qn = work.tile([128, 256], dt.float32, tag="qn", bufs=4)
                nc.gpsimd.tensor_tensor(qn[:].rearrange("p (qk h d) -> p qk h d", qk=2, h=2), m1[:], m2[:], op=ALU.add)
                st["pend"].append((i, qn))

            def finish_prep(st):
                while st["pend"]:
                    flush_one(st)

            def emit_att_unit(st, g, hh):
                p = st["p"]
                h = 2 * p + hh
                off = 64 * hh
                av = psV.tile([65, G], dt.float32, tag="av")
                for kpair in range(8):
                    sp = psS.tile([128, 1024], dt.float32, tag="sp")
                    for half in range(2):
                        ki = kpair * 2 + half
                        nc.tensor.matmul(
                            sp[:, half * 512 : (half + 1) * 512],
                            st["kT"][ki // NG][off : off + 64, (ki % NG) * 128 : (ki % NG + 1) * 128],
                            st["qT"][g][off : off + 64, :],
                            start=True, stop=True,
                        )
                    es = p2e.tile([128, 1024], dt.float32r, tag="es")
                    nc.scalar.activation(es[:], sp[:], AF.Exp, scale=SCALE)
                    for half in range(2):
                        ki = kpair * 2 + half
                        nc.tensor.matmul(
                            av[:],
                            vA[ki // NG][:, ki % NG, h, :],
                            es[:, half * 512 : (half + 1) * 512],
                            start=(ki == 0), stop=(ki == NT - 1),
                        )
                rd = den.tile([1, G], dt.float32, tag="rd")
                nc.vector.reciprocal(rd[:], av[64:65, :])
                bc = den.tile([64, G], dt.float32, tag="bc")
                nc.gpsimd.partition_broadcast(bc[:], rd[:], channels=64)
                nc.vector.tensor_tensor(
                    oT[p][g][off : off + 64, :],
                    av[0:64, :], bc[:], op=ALU.mult,
                )

            def emit_proj_tile(i):
                p512 = psA.tile([128, 512], dt.float32, tag="qkv")
                p256 = psA.tile([128, 256], dt.float32, tag="qkv")
                for pp_ in range(3):
                    st_, spp = (pp_ == 0), (pp_ == 2)
                    sl = oT[pp_][i // NG][:, (i % NG) * 128 : (i % NG + 1) * 128]
                    nc.tensor.matmul(p512[:], sl, prW[pp_][:, 0:512], start=st_, stop=spp)
                    nc.tensor.matmul(p256[:], sl, prW[pp_][:, 512:768], start=st_, stop=spp)
                os_ = outp.tile([128, C], dt.float32, tag="os")
                nc.vector.tensor_copy(os_[:, 0:512], p512[:])
                nc.vector.tensor_copy(os_[:, 512:768], p256[:])
                nc.sync.dma_start(out[i * 128 : (i + 1) * 128, :], os_[:])

            # pair-0 prep up front (V matmuls included)
            cur = new_pair_state(0)
            for _ in range(NT):
                emit_prep_tile(cur)
            finish_prep(cur)

            outp = None
            proj_queue = list(range(NT))
            for p in range(3):
                nxt = new_pair_state(p + 1) if p < 2 else None
                if p == 2:
                    # x^T / qkv weights are dead after pair-2 prep; free them
                    # and open the output pool so projection can interleave.
                    xw_cm.__exit__(None, None, None)
                    outp_cm = tc.tile_pool(name="outp", bufs=2)
                    outp = outp_cm.__enter__()
                for g in range(NG):
                    for hh in range(2):
                        emit_att_unit(cur, g, hh)
                        if nxt is not None:
                            emit_prep_tile(nxt)
                            emit_prep_tile(nxt)
                        else:
                            budget = 2
                            while budget and proj_queue and proj_queue[0] < g * NG:
                                emit_proj_tile(proj_queue.pop(0))
                                budget -= 1
                if nxt is not None:
                    finish_prep(nxt)
                    cur = nxt
            for i in proj_queue:
                emit_proj_tile(i)
            outp_cm.__exit__(None, None, None)

            if DEBUG_DUMP:
                for g in range(NG):
                    nc.sync.dma_start(dbg_oT0[:, g * G : (g + 1) * G], oT[0][g][:].bitcast(dt.float32))
                for kg in range(NG):
                    nc.sync.dma_start(dbg_vA[:, kg * 4 * HPC * (D + 1) : (kg + 1) * 4 * HPC * (D + 1)],
                                      vA[kg][:].bitcast(dt.float32).rearrange("p a b c -> p (a b c)"))

    nc.compile()
    return nc


_NC = None


def _get_nc():
    global _NC
    if _NC is None:
        _NC = build_program()
    return _NC


def _prep_inputs(x, cos, sin, qkv_w, q_norm_w, k_norm_w, proj_w):
    cos2 = np.asarray(cos, np.float32).reshape(N, D // 2)
    sin2 = np.asarray(sin, np.float32).reshape(N, D // 2)
    cos_full = np.concatenate([cos2, cos2], axis=1)          # [N, 64]
    sin_signed = np.concatenate([-sin2, sin2], axis=1)       # [N, 64]

    def tables(w):
        w = np.asarray(w, np.float32)
        wswap = np.concatenate([w[D // 2 :], w[: D // 2]])
        cw = (8.0 * cos_full * w[None, :]).astype(np.float32)
        sw = (8.0 * sin_signed * wswap[None, :]).astype(np.float32)
        return np.ascontiguousarray(cw), np.ascontiguousarray(sw)

    cwq_, swq_ = tables(q_norm_w)
    cwk_, swk_ = tables(k_norm_w)
    cqk_ = np.ascontiguousarray(np.stack([cwq_, cwk_], axis=1).reshape(N, 2 * D))
    sqk_ = np.ascontiguousarray(np.stack([swq_, swk_], axis=1).reshape(N, 2 * D))

    in_maps = []
    for c in range(8):
        b, hg = c // 2, c % 2
        h0 = HPC * hg
        rows = np.r_[h0 * D : (h0 + HPC) * D]
        wq = qkv_w[rows]          # [384, C]
        wk = qkv_w[C + rows]
        wv = qkv_w[2 * C + rows]
        # pack as [q0|k0, q1|k1, q2|k2, v]
        parts = []
        for p in range(3):
            parts.append(wq[p * 128 : (p + 1) * 128])
            parts.append(wk[p * 128 : (p + 1) * 128])
        parts.append(wv)
        wqkvT_ = np.ascontiguousarray(np.concatenate(parts, 0).T, np.float32)
        projT_ = np.ascontiguousarray(proj_w[:, rows].T, np.float32)
        xT_ = np.ascontiguousarray(x[b].T, np.float32)
        in_maps.append({
            "xT": xT_, "wqkvT": wqkvT_, "projT": projT_,
            "cqk": cqk_, "sqk": sqk_,
        })
    return in_maps


def kernel(x, cos, sin, qkv_w, q_norm_w, k_norm_w, proj_w, proj_b, _want_trace=False):
    x = np.asarray(x, np.float32)
    qkv_w = np.asarray(qkv_w, np.float32)
    proj_w = np.asarray(proj_w, np.float32)
    proj_b = np.asarray(proj_b, np.float32)
    in_maps = _prep_inputs(x, cos, sin, qkv_w, q_norm_w, k_norm_w, proj_w)
    nc = _get_nc()
    res = run_bass_kernel_spmd(nc, in_maps, core_ids=list(range(8)), trace=_want_trace)
    out = np.empty((B, N, C), np.float32)
    for b in range(B):
        out[b] = res.results[2 * b]["out"] + res.results[2 * b + 1]["out"] + proj_b[None, :]
    if _want_trace:
        return out, res
    return out


# revision 38
# speedup vs baseline: 239.0073x; 1.0021x over previous
"""Trainium2 Bass kernel for nn_Attention (B=4, N=2048, C=768, H=12, D=64).

Sharding: core c -> batch b=c//2, head-group hg=c%2 (6 heads each).
qkv_w column-parallel, proj_w row-parallel (host sums the 2 partials per b).
All matmuls in float32r (TF32-like, full PE rate for moving dims >= 256).

Structure (single TileContext program per core), organized so attention for
head-pair p overlaps QK prep for pair p+1:
  - V matmuls for all token tiles first (vA tile, ones column folded in for
    the softmax denominator).
  - Per head-pair p: QK matmuls (host packs wqkvT as [q0k0|q1k1|q2k2|v] so
    each pair is a contiguous 256-wide moving operand), RMSNorm+RoPE via
    host-prepped tables (norm weights and the x8 factor folded in),
    PE-transpose to feature-major; then attention for that pair:
    S^T = K Q^T, exp on ACT (scale=1/8; no max-subtraction needed since
    RMSNorm gives ||q||=||k||=8 so |s|<=8), AV accumulation with the ones
    column giving the denominator; reciprocal + GPSIMD partition_broadcast
    to normalize.
  - Projection per token tile at the end (overlaps the attention tail).
"""
import sys

sys.path.insert(0, "/opt/trn_rl_repo")

import numpy as np
import concourse.bass as bass
import concourse.mybir as mybir
import concourse.tile as tile
from concourse import bacc
from concourse.bass_utils import run_bass_kernel_spmd
from concourse.masks import make_identity

dt = mybir.dt
AF = mybir.ActivationFunctionType
ALU = mybir.AluOpType
AX = mybir.AxisListType

B, N, C = 4, 2048, 768
H, D = 12, 64
HPC = 6            # heads per core
EPS = 1e-6
NT = N // 128      # 16 token tiles
NCHUNK = C // 128  # 6 contraction chunks
SCALE = D ** -0.5  # 0.125
NG = 4             # qi groups
G = N // NG        # 512 per group

DEBUG_DUMP = False


def _bc(ap, idx, count):
    """Insert a broadcast (step 0) free dim at position idx of an AP."""
    a = list(ap.ap)
    a.insert(idx, [0, count])
    return bass.AP(tensor=ap.tensor, offset=ap.offset, ap=a)


def build_program():
    nc = bacc.Bacc(None, target_bir_lowering=False)

    xT = nc.dram_tensor("xT", [C, N], dt.float32r, kind="ExternalInput")
    # host layout: [q0|k0 (256) | q1|k1 | q2|k2 | v (384)]
    wqkvT = nc.dram_tensor("wqkvT", [C, 3 * HPC * D], dt.float32r, kind="ExternalInput")
    projT = nc.dram_tensor("projT", [HPC * D, C], dt.float32r, kind="ExternalInput")
    cqk = nc.dram_tensor("cqk", [N, 2 * D], dt.float32, kind="ExternalInput")
    sqk = nc.dram_tensor("sqk", [N, 2 * D], dt.float32, kind="ExternalInput")
    out = nc.dram_tensor("out", [N, C], dt.float32, kind="ExternalOutput")
    if DEBUG_DUMP:
        dbg_qT0 = nc.dram_tensor("dbg_qT0", [128, N], dt.float32, kind="ExternalOutput")
        dbg_kT0 = nc.dram_tensor("dbg_kT0", [128, N], dt.float32, kind="ExternalOutput")
        dbg_oT0 = nc.dram_tensor("dbg_oT0", [128, N], dt.float32, kind="ExternalOutput")
        dbg_vA = nc.dram_tensor("dbg_vA", [128, NT * HPC * (D + 1)], dt.float32, kind="ExternalOutput")

    with tile.TileContext(nc) as tc:
        with (
            tc.tile_pool(name="persist", bufs=1) as persist,
            tc.tile_pool(name="qkrot", bufs=2) as qkrot,     # qT/kT rotate across pairs
            tc.tile_pool(name="work", bufs=2) as work,
            tc.tile_pool(name="qkblk", bufs=1) as qkblk,
            tc.tile_pool(name="tiny", bufs=2) as tiny,
            tc.tile_pool(name="den", bufs=1) as den,
            tc.tile_pool(name="p2e", bufs=2) as p2e,
            tc.tile_pool(name="psA", bufs=2, space="PSUM") as psA,   # qkv/tp/proj shared slots
            tc.tile_pool(name="psS", bufs=2, space="PSUM") as psS,   # scores (2 banks ea)
            tc.tile_pool(name="psV", bufs=2, space="PSUM") as psV,   # AV accum
        ):
            # ---------------- persistent tiles --------------------------------
            oT = [[persist.tile([128, G], dt.float32r, name=f"oT{p}_{g}", tag=f"oT{p}_{g}")
                   for g in range(NG)] for p in range(3)]
            vA = [persist.tile([128, 4, HPC, D + 1], dt.float32r, name=f"vA{kg}", tag=f"vA{kg}")
                  for kg in range(NG)]
            ident = persist.tile([128, 128], dt.float32, tag="ident")
            make_identity(nc, ident[:])
            ones1 = persist.tile([128, 1], dt.float32, tag="ones1")
            nc.vector.memset(ones1[:], 1.0)
            for kg in range(NG):
                nc.vector.tensor_copy(vA[kg][:, :, :, D : D + 1], _bc(_bc(ones1[:], 1, 4), 2, HPC))

            # weights / x^T / tables
            xw_cm = tc.tile_pool(name="xw", bufs=1)
            xw = xw_cm.__enter__()
            xr = []
            wr = []
            for j in range(NCHUNK):
                xj = xw.tile([128, N], dt.float32r, name=f"xr{j}", tag=f"xr{j}")
                wj = xw.tile([128, 3 * HPC * D], dt.float32r, name=f"wr{j}", tag=f"wr{j}")
                eng = (nc.sync, nc.gpsimd)[j % 2]
                eng.dma_start(xj[:], xT[j * 128 : (j + 1) * 128, :])
                eng.dma_start(wj[:], wqkvT[j * 128 : (j + 1) * 128, :])
                xr.append(xj)
                wr.append(wj)
            tabs = {}
            for name, dram in (("cqk", cqk), ("sqk", sqk)):
                t = persist.tile([128, NT, 2, D], dt.float32, name=name, tag=name)
                nc.gpsimd.dma_start(t[:], dram.rearrange("(t p) (qk d) -> p t qk d", p=128, qk=2))
                tabs[name] = t
            prW = []
            for p in range(3):
                wp = persist.tile([128, C], dt.float32r, name=f"prW{p}", tag=f"prW{p}")
                nc.gpsimd.dma_start(wp[:], projT[p * 128 : (p + 1) * 128, :])
                prW.append(wp)

            # ------- interleaved emission: prep / attention / projection ------
            # Engines execute their instruction streams in order, so emission
            # order IS the schedule. Pair p's attention units interleave the
            # prep-tile emission for pair p+1 (2 tiles per unit) so the PE
            # stream mixes prep matmuls with scores/AV instead of bunching
            # them at pair boundaries. Pair-2 attention interleaves the
            # projection of already-finished qi groups.

            def new_pair_state(p):
                return {
                    "p": p,
                    "qT": [qkrot.tile([128, G], dt.float32r, name=f"qT{p}_{g}", tag=f"qT{g}") for g in range(NG)],
                    "kT": [qkrot.tile([128, G], dt.float32r, name=f"kT{p}_{g}", tag=f"kT{g}") for g in range(NG)],
                    "pend": [],
                    "next": 0,
                }

            def flush_one(st):
                i, qn = st["pend"].pop(0)
                for half, dstT in ((0, st["qT"]), (1, st["kT"])):
                    tp = psA.tile([128, 128], dt.float32, tag="qkv")
                    nc.tensor.transpose(tp[:], qn[:, half * 128 : (half + 1) * 128], ident[:])
                    dst = dstT[i // NG][:, (i % NG) * 128 : (i % NG + 1) * 128]
                    if st["p"] == 0:
                        nc.scalar.copy(dst, tp[:])
                    else:
                        nc.vector.tensor_copy(dst, tp[:])

            def emit_prep_tile(st):
                p = st["p"]
                i = st["next"]
                st["next"] += 1
                if p == 0:
                    vp = psA.tile([128, HPC * D], dt.float32, tag="qkv")
                    for j in range(NCHUNK):
                        nc.tensor.matmul(vp[:], xr[j][:, i * 128 : (i + 1) * 128],
                                         wr[j][:, 6 * 128 : 6 * 128 + HPC * D],
                                         start=(j == 0), stop=(j == NCHUNK - 1))
                    nc.scalar.copy(vA[i // NG][:, i % NG, :, 0:D], vp[:].rearrange("p (h d) -> p h d", h=HPC))
                qkp = psA.tile([128, 256], dt.float32, tag="qkv")
                for j in range(NCHUNK):
                    nc.tensor.matmul(qkp[:], xr[j][:, i * 128 : (i + 1) * 128],
                                     wr[j][:, p * 256 : (p + 1) * 256],
                                     start=(j == 0), stop=(j == NCHUNK - 1))
                if len(st["pend"]) >= 2:
                    flush_one(st)
                qk_sb = qkblk.tile([128, 256], dt.float32, tag=f"qk_sb{i % 4}")
                if p == 0:
                    nc.scalar.copy(qk_sb[:], qkp[:])
                else:
                    nc.vector.tensor_copy(qk_sb[:], qkp[:])
                qk4 = qk_sb[:].rearrange("p (h d) -> p h d", h=4)
                sq = work.tile([128, 4, D], dt.float32, tag="m2")
                nc.vector.tensor_tensor(sq[:], qk4, qk4, op=ALU.mult)
                ss = tiny.tile([128, 4], dt.float32, tag="ss16")
                nc.vector.tensor_reduce(ss[:], sq[:], axis=AX.X, op=ALU.add)
                # rsqrt on DVE (bit-trick + 2 Newton): nf = 1/sqrt(ss+D*EPS)
                ssh = tiny.tile([128, 4], dt.float32, tag="ssh")
                nc.vector.tensor_scalar(ssh[:], ss[:], 0.5, 0.5 * D * EPS,
                                        op0=ALU.mult, op1=ALU.add)
                y0i = tiny.tile([128, 4], dt.int32, tag="y0i")
                nc.vector.tensor_scalar(y0i[:], ss[:].bitcast(dt.int32), 1, 0,
                                        op0=ALU.logical_shift_right, op1=ALU.bitwise_or)
                nc.vector.tensor_scalar(y0i[:], y0i[:], -1, 0x5F3759DF,
                                        op0=ALU.mult, op1=ALU.add)
                nf16 = tiny.tile([128, 4], dt.float32, tag="nf16")
                y1 = tiny.tile([128, 4], dt.float32, tag="y1")
                yw = tiny.tile([128, 4], dt.float32, tag="yw")
                y = y0i[:].bitcast(dt.float32)
                for dst_ in (y1, nf16):
                    nc.vector.tensor_tensor(yw[:], y, y, op=ALU.mult)
                    nc.vector.tensor_tensor(yw[:], yw[:], ssh[:], op=ALU.mult)
                    nc.vector.tensor_scalar(yw[:], yw[:], -1.0, 1.5,
                                            op0=ALU.mult, op1=ALU.add)
                    nc.vector.tensor_tensor(dst_[:], y, yw[:], op=ALU.mult)
                    y = dst_[:]
                nfb = _bc(nf16[:], 2, D)
                t_ = work.tile([128, 4, D], dt.float32, tag="t_")
                nc.vector.tensor_tensor(t_[:], qk4, nfb, op=ALU.mult)
                # tables: [128, NT, 2(qk), D] with heads broadcast
                cwb = _bc(tabs["cqk"][:, i, :, :], 2, 2)
                swb = _bc(tabs["sqk"][:, i, :, :], 2, 2)
                t4 = t_[:].rearrange("p (qk h) d -> p qk h d", qk=2)
                m1 = work.tile([128, 2, 2, D], dt.float32, tag="m1")
                nc.vector.tensor_tensor(m1[:], t4, cwb, op=ALU.mult)
                m2 = work.tile([128, 2, 2, D], dt.float32, tag="m2")
                h_ = D // 2
                nc.gpsimd.tensor_tensor(m2[:, :, :, 0:h_], t4[:, :, :, h_:D], swb[:, :, :, 0:h_], op=ALU.mult)
                nc.gpsimd.tensor_tensor(m2[:, :, :, h_:D], t4[:, :, :, 0:h_], swb[:, :, :, h_:D], op=ALU.mult)
                qn = work.tile([128, 256], dt.float32, tag="qn", bufs=4)
                nc.gpsimd.tensor_tensor(qn[:].rearrange("p (qk h d) -> p qk h d", qk=2, h=2), m1[:], m2[:], op=ALU.add)
                st["pend"].append((i, qn))

            def finish_prep(st):
                while st["pend"]:
                    flush_one(st)

            def emit_att_unit(st, g, hh):
                p = st["p"]
                h = 2 * p + hh
                off = 64 * hh
                av = psV.tile([65, G], dt.float32, tag="av")
                for kpair in range(8):
                    sp = psS.tile([128, 1024], dt.float32, tag="sp")
                    for half in range(2):
                        ki = kpair * 2 + half
                        nc.tensor.matmul(
                            sp[:, half * 512 : (half + 1) * 512],
                            st["kT"][ki // NG][off : off + 64, (ki % NG) * 128 : (ki % NG + 1) * 128],
                            st["qT"][g][off : off + 64, :],
                            start=True, stop=True,
                        )
                    es = p2e.tile([128, 1024], dt.float32r, tag="es")
                    nc.scalar.activation(es[:], sp[:], AF.Exp, scale=SCALE)
                    for half in range(2):
                        ki = kpair * 2 + half
                        nc.tensor.matmul(
                            av[:],
                            vA[ki // NG][:, ki % NG, h, :],
                            es[:, half * 512 : (half + 1) * 512],
                            start=(ki == 0), stop=(ki == NT - 1),
                        )
                rd = den.tile([1, G], dt.float32, tag="rd")
                nc.vector.reciprocal(rd[:], av[64:65, :])
                bc = den.tile([64, G], dt.float32, tag="bc")
                nc.gpsimd.partition_broadcast(bc[:], rd[:], channels=64)
                nc.vector.tensor_tensor(
                    oT[p][g][off : off + 64, :],
                    av[0:64, :], bc[:], op=ALU.mult,
                )

            def emit_proj_tile(i):
                p512 = psA.tile([128, 512], dt.float32, tag="qkv")
                p256 = psA.tile([128, 256], dt.float32, tag="qkv")
                for pp_ in range(3):
                    st_, spp = (pp_ == 0), (pp_ == 2)
                    sl = oT[pp_][i // NG][:, (i % NG) * 128 : (i % NG + 1) * 128]
                    nc.tensor.matmul(p512[:], sl, prW[pp_][:, 0:512], start=st_, stop=spp)
                    nc.tensor.matmul(p256[:], sl, prW[pp_][:, 512:768], start=st_, stop=spp)
                os_ = outp.tile([128, C], dt.float32, tag="os")
                nc.vector.tensor_copy(os_[:, 0:512], p512[:])
                nc.vector.tensor_copy(os_[:, 512:768], p256[:])
                nc.sync.dma_start(out[i * 128 : (i + 1) * 128, :], os_[:])

            # pair-0 prep up front (V matmuls included)
            cur = new_pair_state(0)
            for _ in range(NT):
                emit_prep_tile(cur)
            finish_prep(cur)

            outp = None
            proj_queue = list(range(NT))
            for p in range(3):
                nxt = new_pair_state(p + 1) if p < 2 else None
                if p == 2:
                    # x^T / qkv weights are dead after pair-2 prep; free them
                    # and open the output pool so projection can interleave.
                    xw_cm.__exit__(None, None, None)
                    outp_cm = tc.tile_pool(name="outp", bufs=2)
                    outp = outp_cm.__enter__()
                for g in range(NG):
                    for hh in range(2):
                        emit_att_unit(cur, g, hh)
                        if nxt is not None:
                            emit_prep_tile(nxt)
                            emit_prep_tile(nxt)
                        else:
                            budget = 2
                            while budget and proj_queue and proj_queue[0] < g * NG:
                                emit_proj_tile(proj_queue.pop(0))
                                budget -= 1
                if nxt is not None:
                    finish_prep(nxt)
                    cur = nxt
            for i in proj_queue:
                emit_proj_tile(i)
            outp_cm.__exit__(None, None, None)

            if DEBUG_DUMP:
                for g in range(NG):
                    nc.sync.dma_start(dbg_oT0[:, g * G : (g + 1) * G], oT[0][g][:].bitcast(dt.float32))
                for kg in range(NG):
                    nc.sync.dma_start(dbg_vA[:, kg * 4 * HPC * (D + 1) : (kg + 1) * 4 * HPC * (D + 1)],
                                      vA[kg][:].bitcast(dt.float32).rearrange("p a b c -> p (a b c)"))

    nc.compile()
    return nc


_NC = None


def _get_nc():
    global _NC
    if _NC is None:
        _NC = build_program()
    return _NC


def _prep_inputs(x, cos, sin, qkv_w, q_norm_w, k_norm_w, proj_w):
    cos2 = np.asarray(cos, np.float32).reshape(N, D // 2)
    sin2 = np.asarray(sin, np.float32).reshape(N, D // 2)
    cos_full = np.concatenate([cos2, cos2], axis=1)          # [N, 64]
    sin_signed = np.concatenate([-sin2, sin2], axis=1)       # [N, 64]

    def tables(w):
        w = np.asarray(w, np.float32)
        wswap = np.concatenate([w[D // 2 :], w[: D // 2]])
        cw = (8.0 * cos_full * w[None, :]).astype(np.float32)
        sw = (8.0 * sin_signed * wswap[None, :]).astype(np.float32)
        return np.ascontiguousarray(cw), np.ascontiguousarray(sw)

    cwq_, swq_ = tables(q_norm_w)
    cwk_, swk_ = tables(k_norm_w)
    cqk_ = np.ascontiguousarray(np.stack([cwq_, cwk_], axis=1).reshape(N, 2 * D))
    sqk_ = np.ascontiguousarray(np.stack([swq_, swk_], axis=1).reshape(N, 2 * D))

    in_maps = []
    for c in range(8):
        b, hg = c // 2, c % 2
        h0 = HPC * hg
        rows = np.r_[h0 * D : (h0 + HPC) * D]
        wq = qkv_w[rows]          # [384, C]
        wk = qkv_w[C + rows]
        wv = qkv_w[2 * C + rows]
        # pack as [q0|k0, q1|k1, q2|k2, v]
        parts = []
        for p in range(3):
            parts.append(wq[p * 128 : (p + 1) * 128])
            parts.append(wk[p * 128 : (p + 1) * 128])
        parts.append(wv)
        wqkvT_ = np.ascontiguousarray(np.concatenate(parts, 0).T, np.float32)
        projT_ = np.ascontiguousarray(proj_w[:, rows].T, np.float32)
        xT_ = np.ascontiguousarray(x[b].T, np.float32)
        in_maps.append({
            "xT": xT_, "wqkvT": wqkvT_, "projT": projT_,
            "cqk": cqk_, "sqk": sqk_,
        })
    return in_maps


def kernel(x, cos, sin, qkv_w, q_norm_w, k_norm_w, proj_w, proj_b, _want_trace=False):
    x = np.asarray(x, np.float32)
    qkv_w = np.asarray(qkv_w, np.float32)
    proj_w = np.asarray(proj_w, np.float32)
    proj_b = np.asarray(proj_b, np.float32)
    in_maps = _prep_inputs(x, cos, sin, qkv_w, q_norm_w, k_norm_w, proj_w)
    nc = _get_nc()
    res = run_bass_kernel_spmd(nc, in_maps, core_ids=list(range(8)), trace=_want_trace)
    out = np.empty((B, N, C), np.float32)
    for b in range(B):
        out[b] = res.results[2 * b]["out"] + res.results[2 * b + 1]["out"] + proj_b[None, :]
    if _want_trace:
        return out, res
    return out
